# revision 1
# baseline (speedup 1.0000x reference)
"""Trainium2 Bass kernel for nn_ContConv1dDenseSim (banded continuous conv).

Math (reference):
  dt[b,l,j] = times[b,l]-times[b,j], masked to a causal band j in [l-W+1, l]
  (W = (sim_size+1)*kernel_size = 30), true_ids[b,j], and a row-validity mask.
  h = relu(dt*w1+b1)  (8 hidden), kv = (h@w2+b2) masked, reshaped (16,16)
  out[b,l,o] = sum_{j,i} features[b,j,i] * kv[b,l,j,i,o]

Factorization used here:
  G[b,j,k,o]  = sum_i f[b,j,i] * W2[k,i,o]   (k=0..7), G[b,j,8,o] = f[b,j]@B2
  A_k[j,l]    = band(l-j) * relu(dt[l,j]*w1[k]+b1[k])  (k=0..7), A_8 = band
  out[b,l,o]  = row_valid[l] * sum_{j,k} A_k[j,l] * (true_ids[j]*G[b,j,k,o])

Sharding: 8 cores = 2 batches x 4 query-row blocks of 128. Each core sees a
157-column window (128 + W-1) of keys and produces a (128,16) output block
(returned transposed as (16,128); the host transposes back).

On-device layout: window column index jl on SBUF partitions, query row p on
the free dim. The 157-long window is stored as a 256-wide "folded" pair of
column blocks [jl=0..127 | jl=128..156 (+pad)] sharing partitions, so the
relu/mask stages process one (128,256) tile per hidden channel. The banded
contraction is 18 PSUM-accumulated matmuls (9 channels x 2 K-splits) with the
small G factor stationary.

NOTE: TRN2 engine instructions only encode a single sync-wait, so the program
is ordered so each engine's first touch of any foreign-produced tensor is an
instruction with exactly one new cross-engine dependency (tiny "observer" ops
where needed), and the Tile kernel-tail drain is pre-satisfied by single-wait
SP nops.
"""

import numpy as np
import concourse.bass as bass
import concourse.tile as tile
import concourse.mybir as mybir
from concourse.bass_utils import run_bass_kernel_spmd
from concourse.tile_rust import add_dep_helper

F32 = mybir.dt.float32
Alu = mybir.AluOpType
Act = mybir.ActivationFunctionType

BS, L, CH, HID, KS = 2, 512, 16, 8, 5
LBLK = 128                      # query rows per core
NBLK = L // LBLK                # 4
NCORES = BS * NBLK              # 8
NKP = HID + 1                   # A channels (8 hidden + mask)
NF = NKP * CH                   # 144 G columns
W2 = 2 * LBLK                   # folded window width (256)
NPAR = 3 + 2 * HID              # packed per-partition params columns

# test harness hooks
TRACE = False
LAST = None

_prog_cache = {}


def _build(W):
    """Build the single-core SPMD program for band width W (30 for sim=5)."""
    WIN = LBLK + W - 1          # real window columns (157)
    LO = WIN - 128              # columns in the second fold (29)
    nc = bass.Bass(trn_type="TRN2")

    # [ones; t_win padded to 256] (cols 0:256) | [t_row; -ones] (cols 256:384)
    dtpk = nc.declare_dram_parameter("dtpk", [2, W2 + LBLK], F32,
                                     isOutput=False)
    # feat_win^T padded to 256 | W2p with b2 block (cols 256:400)
    fw = nc.declare_dram_parameter("fw", [CH, W2 + NF], F32,
                                   isOutput=False)
    # col 0: tiw[0:128], col 1: tiw[128:WIN] (padded), col 2: row_valid,
    # cols 3:3+HID: w1 replicated, cols 3+HID:3+2*HID: b1 replicated
    par = nc.declare_dram_parameter("par", [128, NPAR], F32, isOutput=False)
    out_d = nc.declare_dram_parameter("out", [LBLK, CH], F32, isOutput=True)

    with tile.TileContext(nc) as tc:
        with (
            tc.tile_pool(name="sb", bufs=1) as sb,
            tc.tile_pool(name="ps", bufs=1, space="PSUM") as ps,
        ):
            # ---- input loads: two issuing sequencers, one DMA per group ----
            t_dtpk = sb.tile([2, W2 + LBLK], F32)
            dma_a = nc.sync.dma_start(t_dtpk[:], dtpk[:])
            t_fw = sb.tile([CH, W2 + NF], F32)
            dma_b = nc.scalar.dma_start(t_fw[:], fw[:])
            t_par = sb.tile([128, NPAR], F32)
            dma_c = nc.sync.dma_start(t_par[:], par[:])
            tiw_up = t_par[:, 0:1]
            tiw_lo = t_par[0:LO, 1:2]
            rv = t_par[:, 2:3]

            # ---- band mask, folded: [:,0:128] up block, [:,128:256] lo ----
            ones = sb.tile([128, W2], F32)
            nc.vector.memset(ones[:], 1.0)
            btmp = sb.tile([128, W2], F32)
            band = sb.tile([128, W2], F32)
            # up: keep jl - p >= 0  (jl = q)
            nc.gpsimd.affine_select(btmp[:, 0:LBLK], ones[:, 0:LBLK],
                                    [[-1, LBLK]], Alu.is_ge, 0.0,
                                    base=0, channel_multiplier=1)
            # up: keep (W-1) - jl + p >= 0
            nc.gpsimd.affine_select(band[:, 0:LBLK], btmp[:, 0:LBLK],
                                    [[1, LBLK]], Alu.is_ge, 0.0,
                                    base=W - 1, channel_multiplier=-1)
            # lo (jl = 128+q): keep p - (128-(W-1)) - q >= 0
            nc.gpsimd.affine_select(btmp[:, LBLK:W2], ones[:, LBLK:W2],
                                    [[1, LBLK]], Alu.is_ge, 0.0,
                                    base=(W - 1) - 128, channel_multiplier=-1)
            # lo: keep (LO-1) - q >= 0  (zero the fold's padding rows)
            last_gp = nc.gpsimd.affine_select(band[:, LBLK:W2],
                                              btmp[:, LBLK:W2],
                                              [[0, LBLK]], Alu.is_ge, 0.0,
                                              base=LO - 1,
                                              channel_multiplier=-1)

            # ---- observers (single-wait discipline, see module docstring) --
            obs_a = sb.tile([1, 1], F32)
            nc.scalar.activation(obs_a[:], t_par[0:1, 0:1], Act.Copy)
            obs_v = sb.tile([1, 2], F32)
            nc.vector.tensor_copy(obs_v[:, 0:1], t_par[0:1, 0:1])
            nc.vector.tensor_copy(obs_v[:, 1:2], band[0:1, LBLK:LBLK + 1])

            # ---- dtT[jl, p] = t_row[p] - t_win[jl], folded (128,256) ----
            p_dt = ps.tile([128, W2], F32)
            rhs_dt = t_dtpk[:, W2:W2 + LBLK]
            nc.tensor.matmul(p_dt[:, 0:LBLK], t_dtpk[:, 0:LBLK], rhs_dt,
                             start=True, stop=True)
            nc.tensor.matmul(p_dt[:, LBLK:W2], t_dtpk[:, LBLK:W2], rhs_dt,
                             start=True, stop=True)

            # ---- G[jl, k*16+o] = feat_win[jl] @ W2p, folded (128,288) ----
            p_g = ps.tile([128, 2 * NF], F32)
            w2p_s = t_fw[:, W2:W2 + NF]
            nc.tensor.matmul(p_g[:, 0:NF], t_fw[:, 0:LBLK],
                             w2p_s, start=True, stop=True)
            nc.tensor.matmul(p_g[:, NF:2 * NF], t_fw[:, LBLK:W2],
                             w2p_s, start=True, stop=True)
            g_sb = sb.tile([128, 2 * NF], F32)
            nc.vector.tensor_scalar_mul(g_sb[:, 0:NF], p_g[:, 0:NF], tiw_up)
            nc.vector.tensor_scalar_mul(g_sb[0:LO, NF:2 * NF],
                                        p_g[0:LO, NF:2 * NF], tiw_lo)

            # ---- A channels: relu(dt*w1k + b1k) * band, one (128,256)/k ----
            a_full = sb.tile([128, HID * W2], F32)
            last_act = None
            for k in range(HID):
                s = slice(k * W2, (k + 1) * W2)
                last_act = nc.scalar.activation(
                    a_full[:, s], p_dt[:], Act.Relu,
                    bias=t_par[:, 3 + HID + k:4 + HID + k],
                    scale=t_par[:, 3 + k:4 + k])
                nc.vector.tensor_mul(a_full[:, s], a_full[:, s], band[:])

            # ---- out[p, o] = sum_k sum_jl A_k[jl,p] * G[jl,k*16+o] ----
            # A-slices are the stationary side (fp32 LDWEIGHTS streams at
            # 2 cyc/row vs 4 cyc/row matmul); the 16-wide G is the moving
            # tensor, so each pair costs ~LDW only.
            p_out = ps.tile([LBLK, CH], F32)
            last_pe = None

            def a_lhs(k):
                if k < HID:
                    return a_full[:, k * W2:(k + 1) * W2]
                return band[:, :]

            for k in range(NKP):
                nc.tensor.matmul(p_out[:], a_lhs(k)[:, 0:LBLK],
                                 g_sb[:, k * CH:(k + 1) * CH],
                                 start=(k == 0), stop=False)
                last_pe = nc.tensor.matmul(
                    p_out[:], a_lhs(k)[0:LO, LBLK:W2],
                    g_sb[0:LO, NF + k * CH:NF + (k + 1) * CH],
                    start=False, stop=(k == NKP - 1))

            # ---- row-validity fold + store ----
            o_sb = sb.tile([LBLK, CH], F32)
            last_dve = nc.vector.tensor_scalar_mul(o_sb[:], p_out[:], rv)
            dma_o = nc.sync.dma_start(out_d[:], o_sb[:])

            # The Tile kernel-tail drain waits on every outstanding
            # semaphore, but TRN2 instructions encode at most one sync
            # wait. Observe each producer from the SP sequencer with
            # single-wait nops so the drain itself needs none.
            for prod in (dma_a, dma_b, dma_c, dma_o,
                         last_gp, last_act, last_dve, last_pe):
                nop = nc.sync.nop(nofuse=True, hint="predrain_observer")
                add_dep_helper(nop.ins, prod.ins, sync=True,
                               reason="pre-drain single-wait observer")

    heavy = [(nm, type(i).__name__, len(i.sync_info.on_wait))
             for nm, i in nc.inst_map.items()
             if getattr(i, "sync_info", None) is not None
             and i.sync_info.on_wait
             and len(i.sync_info.on_wait) > 1
             and type(i).__name__ != "InstDrain"]
    if heavy:
        raise RuntimeError(f"multi-wait instructions would fail walrus: {heavy}")
    return nc


def kernel(times, features, lengths, true_ids, sim_size, w1, b1, w2, b2):
    global LAST
    times = np.ascontiguousarray(np.asarray(times, dtype=np.float32))
    features = np.ascontiguousarray(np.asarray(features, dtype=np.float32))
    lengths = np.asarray(lengths)
    true_ids = np.asarray(true_ids)
    sim = int(np.asarray(sim_size))
    w1 = np.asarray(w1, dtype=np.float32).reshape(-1)
    b1 = np.asarray(b1, dtype=np.float32).reshape(-1)
    w2 = np.asarray(w2, dtype=np.float32)
    b2 = np.asarray(b2, dtype=np.float32)

    W = (sim + 1) * KS
    WIN = LBLK + W - 1
    LO = WIN - 128

    import os
    raw = bool(os.environ.get("BASS_RAW"))
    key = (W, raw)
    if key not in _prog_cache:
        if raw:
            import kernel_raw
            _prog_cache[key] = kernel_raw.build_raw(W)
        else:
            _prog_cache[key] = _build(W)
    nc = _prog_cache[key]

    w2p = np.concatenate(
        [w2.reshape(HID, CH, CH).transpose(1, 0, 2).reshape(CH, HID * CH),
         b2.reshape(CH, CH)], axis=1).astype(np.float32)

    in_maps = []
    for core in range(NCORES):
        b, blk = divmod(core, NBLK)
        l0 = blk * LBLK
        idx = np.arange(l0 - (W - 1), l0 + LBLK)
        valid = idx >= 0
        idxc = np.clip(idx, 0, L - 1)
        t_win = np.where(valid, times[b, idxc], 0.0).astype(np.float32)
        feat_win = np.where(valid[:, None], features[b, idxc, :], 0.0)
        tiw = (true_ids[b, idxc] & valid).astype(np.float32)
        t_row = times[b, l0:l0 + LBLK].astype(np.float32)
        rv = (np.arange(l0, l0 + LBLK) <=
              (sim + 1) * (int(lengths[b]) - 1)).astype(np.float32)

        dtpk = np.zeros((2, W2 + LBLK), np.float32)
        dtpk[0, :W2] = 1.0
        dtpk[1, :WIN] = t_win
        dtpk[0, W2:] = t_row
        dtpk[1, W2:] = -1.0

        fw = np.zeros((CH, W2 + NF), np.float32)
        fw[:, :WIN] = feat_win.T
        fw[:, W2:W2 + NF] = w2p

        if raw:
            par = np.zeros((128, 21 + LBLK), np.float32)
        else:
            par = np.zeros((128, NPAR), np.float32)
        par[:, 0] = tiw[:128]
        par[:LO, 1] = tiw[128:]
        par[:, 2] = rv
        par[:, 3:3 + HID] = w1[None, :]
        par[:, 3 + HID:3 + 2 * HID] = b1[None, :]
        if raw:
            par[:, 19] = t_win[:128]
            par[:LO, 20] = t_win[128:]
            par[:, 21:] = t_row[None, :]
            in_maps.append({"fw": fw, "par": par})
        else:
            in_maps.append({"dtpk": dtpk, "fw": fw, "par": par})

    res = run_bass_kernel_spmd(nc, in_maps, core_ids=list(range(NCORES)),
                               trace=TRACE)
    LAST = res

    out = np.zeros((BS, L, CH), np.float32)
    for core in range(NCORES):
        b, blk = divmod(core, NBLK)
        out[b, blk * LBLK:(blk + 1) * LBLK, :] = res.results[core]["out"]
    return out



# revision 6
# speedup vs baseline: 1.1926x; 1.1926x over previous
"""Trainium2 Bass kernel for nn_ContConv1dDenseSim (banded continuous conv).

Math (reference):
  dt[b,l,j] = times[b,l]-times[b,j], masked to a causal band j in [l-W+1, l]
  (W = (sim_size+1)*kernel_size = 30), true_ids[b,j], and a row-validity mask.
  h = relu(dt*w1+b1)  (8 hidden), kv = (h@w2+b2) masked, reshaped (16,16)
  out[b,l,o] = sum_{j,i} features[b,j,i] * kv[b,l,j,i,o]

Factorization used here:
  G[b,j,k,o]  = sum_i f'[b,j,i] * W2[k,i,o]  (k=0..7), G[b,j,8,o] = f'[b,j]@B2
                with f' = features * true_ids (host-folded)
  A_k[j,l]    = band(l-j) * relu(dt[l,j]*w1[k]+b1[k]) (k=0..7), A_8 = band
  out[b,l,o]  = row_valid[l] * sum_{j,k} A_k[j,l] * G[b,j,k,o]

Sharding: 8 cores = 2 batches x 4 query-row blocks of 128. Each core sees a
157-column window (128 + W-1) of keys and produces a (128,16) output block.

v2 layout: the host precomputes the (already band-masked) delta-time tile and
the band mask, so the device does no dt matmul and no affine_selects. The
window is stored folded: [jl=0..127 | jl=128..156 (+pad)] share partitions,
query p on the free dim (128 up cols + 128 lo cols). All PE inputs are bf16
(fp32 matmuls cost 2 half-rate passes on TRN2); accumulation stays fp32 in
PSUM. relu channels are split Scalar (6 via activation) / Pool (2 via
tensor_scalar + scalar_tensor_tensor); DVE applies the band mask for the
Scalar channels and converts G from PSUM to bf16.

NOTE: TRN2 engine instructions only encode a single sync-wait, so the program
is ordered so each engine's first touch of any foreign-produced tensor is an
instruction with exactly one new cross-engine dependency (tiny "observer" ops
where needed), and the Tile kernel-tail drain is pre-satisfied by single-wait
SP nops.
"""

import numpy as np
import ml_dtypes
import concourse.bass as bass
import concourse.tile as tile
import concourse.mybir as mybir
from concourse.bass_utils import run_bass_kernel_spmd
from concourse.tile_rust import add_dep_helper

F32 = mybir.dt.float32
BF16 = mybir.dt.bfloat16
NPBF16 = ml_dtypes.bfloat16
Alu = mybir.AluOpType
Act = mybir.ActivationFunctionType

BS, L, CH, HID, KS = 2, 512, 16, 8, 5
LBLK = 128                      # query rows per core
NBLK = L // LBLK                # 4
NCORES = BS * NBLK              # 8
NKP = HID + 1                   # G blocks (8 hidden + band/b2 channel)
NF = NKP * CH                   # 144 G columns per fold
W2F = 2 * LBLK                  # folded window width (256)
NPAR = W2F + 1 + 2 * HID        # dp columns: dt(256) rv(1) w1(8) b1(8)

N_SCAL = 6                      # relu channels on the Scalar engine
# remaining HID - N_SCAL channels run on Pool (gpsimd)

# test harness hooks
TRACE = False
LAST = None

_prog_cache = {}


def _build(W):
    """Build the single-core SPMD program for band width W (30 for sim=5)."""
    WIN = LBLK + W - 1          # real window columns (157)
    LO = WIN - 128              # columns in the second fold (29)
    nc = bass.Bass(trn_type="TRN2")

    # dt folded (band-masked, fp32) | rv | w1 replicated | b1 replicated
    dp = nc.declare_dram_parameter("dp", [128, NPAR], F32, isOutput=False)
    # band mask folded, bf16 {0,1}
    bd = nc.declare_dram_parameter("bd", [128, W2F], BF16, isOutput=False)
    # feat_win^T (true_ids-folded) padded to 256 | w2p blocks (144)
    fw = nc.declare_dram_parameter("fw", [CH, W2F + NF], BF16, isOutput=False)
    out_d = nc.declare_dram_parameter("out", [LBLK, CH], BF16, isOutput=True)

    with tile.TileContext(nc) as tc:
        with (
            tc.tile_pool(name="sb", bufs=1) as sb,
            tc.tile_pool(name="ps", bufs=1, space="PSUM") as ps,
        ):
            t_dp = sb.tile([128, NPAR], F32)
            t_bd = sb.tile([128, W2F], BF16)
            t_fw = sb.tile([CH, W2F + NF], BF16)

            dt_ap = t_dp[:, 0:W2F]
            rv = t_dp[:, W2F:W2F + 1]

            def w1c(k):
                return t_dp[:, W2F + 1 + k:W2F + 2 + k]

            def b1c(k):
                return t_dp[:, W2F + 1 + HID + k:W2F + 2 + HID + k]

            # ---- input DMAs: SP carries fw+dp, Scalar carries bd ----
            dma_fw = nc.sync.dma_start(t_fw[:], fw[:])
            dma_dp = nc.sync.dma_start(t_dp[:], dp[:])
            dma_bd = nc.scalar.dma_start(t_bd[:], bd[:])

            # ---- G = f' @ w2p (PSUM fp32), folded [up 0:144 | lo 144:288] --
            p_g = ps.tile([128, 2 * NF], F32)
            w2p_s = t_fw[:, W2F:W2F + NF]
            mm_gu = nc.tensor.matmul(p_g[:, 0:NF], t_fw[:, 0:LBLK], w2p_s,
                                     start=True, stop=True)
            mm_gl = nc.tensor.matmul(p_g[0:LO, NF:2 * NF],
                                     t_fw[:, LBLK:LBLK + LO], w2p_s,
                                     start=True, stop=True)

            # ---- Scalar: relu channels 0..N_SCAL-1 (ATL overlaps DMAs) ----
            a_full = sb.tile([128, HID * W2F], BF16)

            def a_k(k):
                return a_full[:, k * W2F:(k + 1) * W2F]

            acts = []
            for k in range(N_SCAL):
                acts.append(nc.scalar.activation(
                    a_k(k), dt_ap, Act.Relu, bias=b1c(k), scale=w1c(k)))

            # ---- Pool: relu channels N_SCAL..7 (ts + stt, band fused) ----
            # first-touch observer so stt's band dep doesn't stack on the
            # same-engine pipeline hazard wait
            obs_p = sb.tile([1, 2], BF16)
            nc.gpsimd.tensor_copy(obs_p[:, 0:1], t_bd[0:1, 0:1])
            y_pool = sb.tile([128, (HID - N_SCAL) * W2F], BF16)
            pool_ops = []
            for i, k in enumerate(range(N_SCAL, HID)):
                ys = y_pool[:, i * W2F:(i + 1) * W2F]
                nc.gpsimd.tensor_scalar(ys, dt_ap, w1c(k), b1c(k),
                                        Alu.mult, Alu.add)
                nc.gpsimd.tensor_scalar_max(ys, ys, 0.0)
                pool_ops.append(nc.gpsimd.tensor_tensor(
                    a_k(k), ys, t_bd[:], Alu.mult))

            # ---- DVE: G psum->sbuf bf16, band-mask the Scalar channels ----
            g_sb = sb.tile([128, 2 * NF], BF16)
            cp_gu = nc.vector.tensor_copy(g_sb[:, 0:NF], p_g[:, 0:NF])
            cp_gl = nc.vector.tensor_copy(g_sb[0:LO, NF:2 * NF],
                                          p_g[0:LO, NF:2 * NF])
            obs = sb.tile([1, 2], BF16)
            nc.vector.tensor_copy(obs[:, 0:1], t_bd[0:1, 0:1])
            # observe dp from DVE so rv_mul's DMA dep is pre-satisfied
            obs_dp = sb.tile([1, 1], F32)
            nc.vector.tensor_copy(obs_dp[:], t_dp[0:1, 0:1])
            masks = []
            for k in range(N_SCAL):
                masks.append(nc.vector.tensor_tensor(
                    a_k(k), a_k(k), t_bd[:], Alu.mult))

            # ---- banded contraction: accumulate 9 channels x 2 folds ----
            p_out = ps.tile([LBLK, CH], F32)

            def ch_up(k, lhsT, start=False):
                return nc.tensor.matmul(
                    p_out[:], lhsT[:, 0:LBLK], g_sb[:, k * CH:(k + 1) * CH],
                    start=start, stop=False)

            def ch_lo(k, lhsT, stop=False):
                return nc.tensor.matmul(
                    p_out[:], lhsT[0:LO, LBLK:W2F],
                    g_sb[0:LO, NF + k * CH:NF + (k + 1) * CH],
                    start=False, stop=stop)

            # band channel first (ready earliest), then channels in the
            # order their producers finish: pool ch6, scalar ch0..,
            # pool ch7 slotted mid-stream.
            ch_up(HID, t_bd, start=True)
            ch_lo(HID, t_bd)
            order = [N_SCAL, 0, 1, N_SCAL + 1, 2, 3, 4, 5]
            order = [k for k in order if k < HID] + \
                    [k for k in range(HID) if k not in order]
            last_pe = None
            for k in order:
                ch_up(k, a_k(k))
                last_pe = ch_lo(k, a_k(k), stop=(k == order[-1]))

            # ---- row-validity fold + store ----
            o_sb = sb.tile([LBLK, CH], BF16)
            rv_mul = nc.vector.tensor_scalar_mul(o_sb[:], p_out[:], rv)
            dma_o = nc.sync.dma_start(out_d[:], o_sb[:])

            # The Tile kernel-tail drain waits on every outstanding
            # semaphore, but TRN2 instructions encode at most one sync
            # wait. Observe each producer from the SP sequencer with
            # single-wait nops so the drain itself needs none.
            for prod in (dma_fw, dma_dp, dma_bd, dma_o,
                         acts[-1], pool_ops[-1], rv_mul, last_pe):
                nop = nc.sync.nop(nofuse=True, hint="predrain_observer")
                add_dep_helper(nop.ins, prod.ins, sync=True,
                               reason="pre-drain single-wait observer")

    heavy = [(nm, type(i).__name__, len(i.sync_info.on_wait))
             for nm, i in nc.inst_map.items()
             if getattr(i, "sync_info", None) is not None
             and i.sync_info.on_wait
             and len(i.sync_info.on_wait) > 1
             and type(i).__name__ != "InstDrain"]
    if heavy:
        raise RuntimeError(f"multi-wait instructions would fail walrus: {heavy}")
    return nc


def kernel(times, features, lengths, true_ids, sim_size, w1, b1, w2, b2):
    global LAST
    times = np.ascontiguousarray(np.asarray(times, dtype=np.float32))
    features = np.ascontiguousarray(np.asarray(features, dtype=np.float32))
    lengths = np.asarray(lengths)
    true_ids = np.asarray(true_ids)
    sim = int(np.asarray(sim_size))
    w1 = np.asarray(w1, dtype=np.float32).reshape(-1)
    b1 = np.asarray(b1, dtype=np.float32).reshape(-1)
    w2 = np.asarray(w2, dtype=np.float32)
    b2 = np.asarray(b2, dtype=np.float32)

    W = (sim + 1) * KS
    WIN = LBLK + W - 1
    LO = WIN - 128

    if W not in _prog_cache:
        _prog_cache[W] = _build(W)
    nc = _prog_cache[W]

    # w2 blocks [i, k*16+o] then b2 block as block 8
    w2p = np.concatenate(
        [w2.reshape(HID, CH, CH).transpose(1, 0, 2).reshape(CH, HID * CH),
         b2.reshape(CH, CH)], axis=1).astype(np.float32)

    # folded band geometry, shared across cores: band_up[jl, p] for
    # jl-p in [0, W-1]; band_lo[q, p] (jl = 128+q) for p >= 128+q-(W-1)
    jl = np.arange(128)[:, None]
    pp = np.arange(128)[None, :]
    band_up = ((pp >= jl - (W - 1)) & (pp <= jl)).astype(np.float32)
    band_lo = np.zeros((128, 128), np.float32)
    band_lo[:LO] = (pp >= (128 - (W - 1)) + jl[:LO]).astype(np.float32)
    band_f32 = np.concatenate([band_up, band_lo], axis=1)
    bd_host = band_f32.astype(NPBF16)

    in_maps = []
    for core in range(NCORES):
        b, blk = divmod(core, NBLK)
        l0 = blk * LBLK
        idx = np.arange(l0 - (W - 1), l0 + LBLK)
        valid = idx >= 0
        idxc = np.clip(idx, 0, L - 1)
        t_win = np.where(valid, times[b, idxc], 0.0).astype(np.float32)
        tiw = (true_ids[b, idxc] & valid).astype(np.float32)
        f_mask = features[b, idxc, :] * tiw[:, None]
        t_row = times[b, l0:l0 + LBLK].astype(np.float32)
        rv = (np.arange(l0, l0 + LBLK) <=
              (sim + 1) * (int(lengths[b]) - 1)).astype(np.float32)

        # dt folded, band-masked
        dt = t_row[None, :] - t_win[:, None]        # [WIN(jl), 128(p)]
        dt_fold = np.zeros((128, W2F), np.float32)
        dt_fold[:, 0:128] = dt[0:128]
        dt_fold[:LO, 128:256] = dt[128:WIN]
        dt_fold *= band_f32

        dp = np.zeros((128, NPAR), np.float32)
        dp[:, 0:W2F] = dt_fold
        dp[:, W2F] = rv
        dp[:, W2F + 1:W2F + 1 + HID] = w1[None, :]
        dp[:, W2F + 1 + HID:W2F + 1 + 2 * HID] = b1[None, :]

        fw = np.zeros((CH, W2F + NF), np.float32)
        fw[:, :WIN] = f_mask.T
        fw[:, W2F:W2F + NF] = w2p

        in_maps.append({"dp": dp, "bd": bd_host,
                        "fw": fw.astype(NPBF16)})

    res = run_bass_kernel_spmd(nc, in_maps, core_ids=list(range(NCORES)),
                               trace=TRACE)
    LAST = res

    out = np.zeros((BS, L, CH), np.float32)
    for core in range(NCORES):
        b, blk = divmod(core, NBLK)
        out[b, blk * LBLK:(blk + 1) * LBLK, :] = \
            res.results[core]["out"].astype(np.float32)
    return out


# revision 7
# speedup vs baseline: 1.5838x; 1.3280x over previous
"""Trainium2 Bass kernel for nn_ContConv1dDenseSim (banded continuous conv).

Math (reference):
  dt[b,l,j] = times[b,l]-times[b,j], masked to a causal band j in [l-W+1, l]
  (W = (sim_size+1)*kernel_size = 30), true_ids[b,j], and a row-validity mask.
  h = relu(dt*w1+b1)  (8 hidden), kv = (h@w2+b2) masked, reshaped (16,16)
  out[b,l,o] = sum_{j,i} features[b,j,i] * kv[b,l,j,i,o]

Factorization used here:
  G[b,j,k,o]  = sum_i f'[b,j,i] * W2[k,i,o]  (k=0..7), G[b,j,8,o] = f'[b,j]@B2
                with f' = features * true_ids (host-folded)
  A_k[j,l]    = band(l-j) * relu(dt[l,j]*w1[k]+b1[k]) (k=0..7), A_8 = band
  out[b,l,o]  = row_valid[l] * sum_{j,k} A_k[j,l] * G[b,j,k,o]

Sharding: 8 cores = 2 batches x 4 query-row blocks of 128. Each core sees a
157-column window (128 + W-1) of keys and produces a (128,16) output block.

v2 layout: the host precomputes the (already band-masked) delta-time tile and
the band mask, so the device does no dt matmul and no affine_selects. The
window is stored folded: [jl=0..127 | jl=128..156 (+pad)] share partitions,
query p on the free dim (128 up cols + 128 lo cols). All PE inputs are bf16
(fp32 matmuls cost 2 half-rate passes on TRN2); accumulation stays fp32 in
PSUM. relu channels are split Scalar (6 via activation) / Pool (2 via
tensor_scalar + scalar_tensor_tensor); DVE applies the band mask for the
Scalar channels and converts G from PSUM to bf16.

NOTE: TRN2 engine instructions only encode a single sync-wait, so the program
is ordered so each engine's first touch of any foreign-produced tensor is an
instruction with exactly one new cross-engine dependency (tiny "observer" ops
where needed), and the Tile kernel-tail drain is pre-satisfied by single-wait
SP nops.
"""

import numpy as np
import ml_dtypes
import concourse.bass as bass
import concourse.tile as tile
import concourse.mybir as mybir
from concourse.bass_utils import run_bass_kernel_spmd
from concourse.tile_rust import add_dep_helper

F32 = mybir.dt.float32
BF16 = mybir.dt.bfloat16
NPBF16 = ml_dtypes.bfloat16
Alu = mybir.AluOpType
Act = mybir.ActivationFunctionType

BS, L, CH, HID, KS = 2, 512, 16, 8, 5
LBLK = 128                      # query rows per core
NBLK = L // LBLK                # 4
NCORES = BS * NBLK              # 8
NKP = HID + 2                   # G blocks (8 hidden + ones/B2 + M2/W2c9)
NF = NKP * CH                   # 160 G columns per fold
W2F = 2 * LBLK                  # folded window width (256)
NPAR = W2F + 1 + 2 * HID        # dp columns: dt(256) rv(1) w1(8) b1(8)


# test harness hooks
TRACE = False
LAST = None

_prog_cache = {}


def _build(W):
    """Build the single-core SPMD program for band width W (30 for sim=5)."""
    WIN = LBLK + W - 1          # real window columns (157)
    LO = WIN - 128              # columns in the second fold (29)
    nc = bass.Bass(trn_type="TRN2")

    # dt folded (band-masked, fp32) | rv | w1 replicated | b1 replicated
    dp = nc.declare_dram_parameter("dp", [128, NPAR], F32, isOutput=False)
    # correction masks folded, bf16: [M2 = band-1 | ones]
    bd = nc.declare_dram_parameter("bd", [128, 2 * W2F], BF16, isOutput=False)
    # feat_win^T (true_ids-folded) padded to 256 | w2p blocks (144)
    fw = nc.declare_dram_parameter("fw", [CH, W2F + NF], BF16, isOutput=False)
    out_d = nc.declare_dram_parameter("out", [LBLK, CH], BF16, isOutput=True)

    with tile.TileContext(nc) as tc:
        with (
            tc.tile_pool(name="sb", bufs=1) as sb,
            tc.tile_pool(name="ps", bufs=1, space="PSUM") as ps,
        ):
            t_dp = sb.tile([128, NPAR], F32)
            t_bd = sb.tile([128, 2 * W2F], BF16)
            t_fw = sb.tile([CH, W2F + NF], BF16)

            dt_ap = t_dp[:, 0:W2F]
            rv = t_dp[:, W2F:W2F + 1]

            def w1c(k):
                return t_dp[:, W2F + 1 + k:W2F + 2 + k]

            def b1c(k):
                return t_dp[:, W2F + 1 + HID + k:W2F + 2 + HID + k]

            # ---- input DMAs: SP carries fw+dp, Scalar carries bd ----
            dma_fw = nc.sync.dma_start(t_fw[:], fw[:])
            dma_dp = nc.sync.dma_start(t_dp[:], dp[:])
            dma_bd = nc.scalar.dma_start(t_bd[:], bd[:])

            # ---- G = f' @ w2p (PSUM fp32), folded [up 0:144 | lo 144:288] --
            p_g = ps.tile([128, 2 * NF], F32)
            w2p_s = t_fw[:, W2F:W2F + NF]
            mm_gu = nc.tensor.matmul(p_g[:, 0:NF], t_fw[:, 0:LBLK], w2p_s,
                                     start=True, stop=True)
            mm_gl = nc.tensor.matmul(p_g[0:LO, NF:2 * NF],
                                     t_fw[:, LBLK:LBLK + LO], w2p_s,
                                     start=True, stop=True)

            # ---- Scalar: all 8 relu channels, unmasked (ATL overlaps) ----
            # off-band dt is 0, so a_k = relu(b1k) there; the ones + M2
            # channels cancel that constant exactly (host folds
            # bf16-rounded relu(b1) into the W2c9 block).
            a_full = sb.tile([128, HID * W2F], BF16)

            def a_k(k):
                return a_full[:, k * W2F:(k + 1) * W2F]

            acts = []
            for k in range(HID):
                acts.append(nc.scalar.activation(
                    a_k(k), dt_ap, Act.Relu, bias=b1c(k), scale=w1c(k)))

            # ---- DVE: G psum->sbuf bf16 ----
            g_sb = sb.tile([128, 2 * NF], BF16)
            cp_gu = nc.vector.tensor_copy(g_sb[:, 0:NF], p_g[:, 0:NF])
            cp_gl = nc.vector.tensor_copy(g_sb[0:LO, NF:2 * NF],
                                          p_g[0:LO, NF:2 * NF])
            # observe dp from DVE so rv_mul's DMA dep is pre-satisfied
            obs_dp = sb.tile([1, 1], F32)
            nc.vector.tensor_copy(obs_dp[:], t_dp[0:1, 0:1])

            # ---- banded contraction: accumulate 9 channels x 2 folds ----
            p_out = ps.tile([LBLK, CH], F32)

            def ch_up(k, lhsT, start=False):
                return nc.tensor.matmul(
                    p_out[:], lhsT[:, 0:LBLK], g_sb[:, k * CH:(k + 1) * CH],
                    start=start, stop=False)

            def ch_lo(k, lhsT, stop=False):
                return nc.tensor.matmul(
                    p_out[:], lhsT[0:LO, LBLK:W2F],
                    g_sb[0:LO, NF + k * CH:NF + (k + 1) * CH],
                    start=False, stop=stop)

            # correction channels first (ready as soon as bd + g land):
            # ones channel (block 8) then M2 = band-1 channel (block 9)
            ones_ap = t_bd[:, W2F:2 * W2F]
            m2_ap = t_bd[:, 0:W2F]
            ch_up(HID, ones_ap, start=True)
            ch_lo(HID, ones_ap)
            ch_up(HID + 1, m2_ap)
            ch_lo(HID + 1, m2_ap)
            last_pe = None
            for k in range(HID):
                ch_up(k, a_k(k))
                last_pe = ch_lo(k, a_k(k), stop=(k == HID - 1))

            # ---- row-validity fold + store ----
            o_sb = sb.tile([LBLK, CH], BF16)
            rv_mul = nc.vector.tensor_scalar_mul(o_sb[:], p_out[:], rv)
            dma_o = nc.sync.dma_start(out_d[:], o_sb[:])

            # The Tile kernel-tail drain waits on every outstanding
            # semaphore, but TRN2 instructions encode at most one sync
            # wait. Observe each producer from the SP sequencer with
            # single-wait nops so the drain itself needs none.
            for prod in (dma_fw, dma_dp, dma_bd, dma_o,
                         acts[-1], rv_mul, last_pe):
                nop = nc.sync.nop(nofuse=True, hint="predrain_observer")
                add_dep_helper(nop.ins, prod.ins, sync=True,
                               reason="pre-drain single-wait observer")

    heavy = [(nm, type(i).__name__, len(i.sync_info.on_wait))
             for nm, i in nc.inst_map.items()
             if getattr(i, "sync_info", None) is not None
             and i.sync_info.on_wait
             and len(i.sync_info.on_wait) > 1
             and type(i).__name__ != "InstDrain"]
    if heavy:
        raise RuntimeError(f"multi-wait instructions would fail walrus: {heavy}")
    return nc


def kernel(times, features, lengths, true_ids, sim_size, w1, b1, w2, b2):
    global LAST
    times = np.ascontiguousarray(np.asarray(times, dtype=np.float32))
    features = np.ascontiguousarray(np.asarray(features, dtype=np.float32))
    lengths = np.asarray(lengths)
    true_ids = np.asarray(true_ids)
    sim = int(np.asarray(sim_size))
    w1 = np.asarray(w1, dtype=np.float32).reshape(-1)
    b1 = np.asarray(b1, dtype=np.float32).reshape(-1)
    w2 = np.asarray(w2, dtype=np.float32)
    b2 = np.asarray(b2, dtype=np.float32)

    W = (sim + 1) * KS
    WIN = LBLK + W - 1
    LO = WIN - 128

    if W not in _prog_cache:
        _prog_cache[W] = _build(W)
    nc = _prog_cache[W]

    # w2 blocks [i, k*16+o], then B2 (ones channel), then W2c9 (M2 channel)
    ck = np.maximum(b1, 0).astype(NPBF16).astype(np.float32)
    W2k = w2.reshape(HID, CH, CH)
    B2m = b2.reshape(CH, CH)
    W2c9 = (ck[:, None, None] * W2k).sum(0) + B2m
    w2p = np.concatenate(
        [W2k.transpose(1, 0, 2).reshape(CH, HID * CH), B2m, W2c9],
        axis=1).astype(np.float32)

    # folded band geometry, shared across cores: band_up[jl, p] for
    # jl-p in [0, W-1]; band_lo[q, p] (jl = 128+q) for p >= 128+q-(W-1)
    jl = np.arange(128)[:, None]
    pp = np.arange(128)[None, :]
    band_up = ((pp >= jl - (W - 1)) & (pp <= jl)).astype(np.float32)
    band_lo = np.zeros((128, 128), np.float32)
    band_lo[:LO] = (pp >= (128 - (W - 1)) + jl[:LO]).astype(np.float32)
    band_f32 = np.concatenate([band_up, band_lo], axis=1)
    ones_f = np.ones((128, 128), np.float32)
    ones_lo = np.zeros((128, 128), np.float32)
    ones_lo[:LO] = 1.0
    bd_host = np.concatenate(
        [band_up - 1.0, band_lo - 1.0, ones_f, ones_lo],
        axis=1).astype(NPBF16)

    in_maps = []
    for core in range(NCORES):
        b, blk = divmod(core, NBLK)
        l0 = blk * LBLK
        idx = np.arange(l0 - (W - 1), l0 + LBLK)
        valid = idx >= 0
        idxc = np.clip(idx, 0, L - 1)
        t_win = np.where(valid, times[b, idxc], 0.0).astype(np.float32)
        tiw = (true_ids[b, idxc] & valid).astype(np.float32)
        f_mask = features[b, idxc, :] * tiw[:, None]
        t_row = times[b, l0:l0 + LBLK].astype(np.float32)
        rv = (np.arange(l0, l0 + LBLK) <=
              (sim + 1) * (int(lengths[b]) - 1)).astype(np.float32)

        # dt folded, band-masked
        dt = t_row[None, :] - t_win[:, None]        # [WIN(jl), 128(p)]
        dt_fold = np.zeros((128, W2F), np.float32)
        dt_fold[:, 0:128] = dt[0:128]
        dt_fold[:LO, 128:256] = dt[128:WIN]
        dt_fold *= band_f32

        dp = np.zeros((128, NPAR), np.float32)
        dp[:, 0:W2F] = dt_fold
        dp[:, W2F] = rv
        dp[:, W2F + 1:W2F + 1 + HID] = w1[None, :]
        dp[:, W2F + 1 + HID:W2F + 1 + 2 * HID] = b1[None, :]

        fw = np.zeros((CH, W2F + NF), np.float32)
        fw[:, :WIN] = f_mask.T
        fw[:, W2F:W2F + NF] = w2p

        in_maps.append({"dp": dp, "bd": bd_host,
                        "fw": fw.astype(NPBF16)})

    res = run_bass_kernel_spmd(nc, in_maps, core_ids=list(range(NCORES)),
                               trace=TRACE)
    LAST = res

    out = np.zeros((BS, L, CH), np.float32)
    for core in range(NCORES):
        b, blk = divmod(core, NBLK)
        out[b, blk * LBLK:(blk + 1) * LBLK, :] = \
            res.results[core]["out"].astype(np.float32)
    return out


# revision 9
# speedup vs baseline: 1.6842x; 1.0634x over previous
"""Trainium2 Bass kernel for nn_ContConv1dDenseSim (banded continuous conv).

Math (reference):
  dt[b,l,j] = times[b,l]-times[b,j], masked to a causal band j in [l-W+1, l]
  (W = (sim_size+1)*kernel_size = 30), true_ids[b,j], and a row-validity mask.
  h = relu(dt*w1+b1)  (8 hidden), kv = (h@w2+b2) masked, reshaped (16,16)
  out[b,l,o] = sum_{j,i} features[b,j,i] * kv[b,l,j,i,o]

Factorization used here:
  G[b,j,k,o]  = sum_i f'[b,j,i] * W2[k,i,o]  (k=0..7), G[b,j,8,o] = f'[b,j]@B2
                with f' = features * true_ids (host-folded)
  A_k[j,l]    = band(l-j) * relu(dt[l,j]*w1[k]+b1[k]) (k=0..7), A_8 = band
  out[b,l,o]  = row_valid[l] * sum_{j,k} A_k[j,l] * G[b,j,k,o]

Sharding: 8 cores = 2 batches x 4 query-row blocks of 128. Each core sees a
157-column window (128 + W-1) of keys and produces a (128,16) output block.

v2 layout: the host precomputes the (already band-masked) delta-time tile and
the band mask, so the device does no dt matmul and no affine_selects. The
window is stored folded: [jl=0..127 | jl=128..156 (+pad)] share partitions,
query p on the free dim (128 up cols + 128 lo cols). All PE inputs are bf16
(fp32 matmuls cost 2 half-rate passes on TRN2); accumulation stays fp32 in
PSUM. relu channels are split Scalar (6 via activation) / Pool (2 via
tensor_scalar + scalar_tensor_tensor); DVE applies the band mask for the
Scalar channels and converts G from PSUM to bf16.

NOTE: TRN2 engine instructions only encode a single sync-wait, so the program
is ordered so each engine's first touch of any foreign-produced tensor is an
instruction with exactly one new cross-engine dependency (tiny "observer" ops
where needed), and the Tile kernel-tail drain is pre-satisfied by single-wait
SP nops.
"""

import numpy as np
import ml_dtypes
import concourse.bass as bass
import concourse.tile as tile
import concourse.mybir as mybir
from concourse.bass_utils import run_bass_kernel_spmd
from concourse.tile_rust import add_dep_helper

F32 = mybir.dt.float32
BF16 = mybir.dt.bfloat16
NPBF16 = ml_dtypes.bfloat16
Alu = mybir.AluOpType
Act = mybir.ActivationFunctionType

BS, L, CH, HID, KS = 2, 512, 16, 8, 5
LBLK = 128                      # query rows per core
NBLK = L // LBLK                # 4
NCORES = BS * NBLK              # 8
NKP = HID + 2                   # G blocks (8 hidden + ones/B2 + M2/W2c9)
NF = NKP * CH                   # 160 G columns per fold
W2F = 2 * LBLK                  # folded window width (256)
NPAR = W2F + 1 + 2 * HID        # dp columns: dt(256) rv(1) w1(8) b1(8)


# test harness hooks
TRACE = False
LAST = None

_prog_cache = {}


def _build(W):
    """Build the single-core SPMD program for band width W (30 for sim=5)."""
    WIN = LBLK + W - 1          # real window columns (157)
    LO = WIN - 128              # columns in the second fold (29)
    nc = bass.Bass(trn_type="TRN2")

    # dt folded (band-masked, fp32) | rv | w1 replicated | b1 replicated
    dp = nc.declare_dram_parameter("dp", [128, NPAR], F32, isOutput=False)
    # correction masks folded, bf16: [M2 = band-1 | ones]
    bd = nc.declare_dram_parameter("bd", [128, 2 * W2F], BF16, isOutput=False)
    # feat_win^T (true_ids-folded) padded to 256 | w2p blocks (144)
    fw = nc.declare_dram_parameter("fw", [CH, W2F + NF], BF16, isOutput=False)
    out_d = nc.declare_dram_parameter("out", [LBLK, CH], BF16, isOutput=True)

    with tile.TileContext(nc) as tc:
        with (
            tc.tile_pool(name="sb", bufs=1) as sb,
            tc.tile_pool(name="ps", bufs=1, space="PSUM") as ps,
        ):
            t_dp = sb.tile([128, NPAR], F32)
            t_bd = sb.tile([128, 2 * W2F], BF16)
            t_fw = sb.tile([CH, W2F + NF], BF16)

            dt_ap = t_dp[:, 0:W2F]
            rv = t_dp[:, W2F:W2F + 1]

            def w1c(k):
                return t_dp[:, W2F + 1 + k:W2F + 2 + k]

            def b1c(k):
                return t_dp[:, W2F + 1 + HID + k:W2F + 2 + HID + k]

            # ---- input DMAs: fw first (small, gates the PE stream) ----
            dma_fw = nc.sync.dma_start(t_fw[:], fw[:])
            dma_dp = nc.sync.dma_start(t_dp[:], dp[:])
            dma_bd = nc.scalar.dma_start(t_bd[:], bd[:])

            # ---- G = f' @ w2p (PSUM fp32), folded [up 0:144 | lo 144:288] --
            p_g = ps.tile([128, 2 * NF], F32)
            w2p_s = t_fw[:, W2F:W2F + NF]
            mm_gu = nc.tensor.matmul(p_g[:, 0:NF], t_fw[:, 0:LBLK], w2p_s,
                                     start=True, stop=True)
            mm_gl = nc.tensor.matmul(p_g[0:LO, NF:2 * NF],
                                     t_fw[:, LBLK:LBLK + LO], w2p_s,
                                     start=True, stop=True)

            # ---- Scalar: all 8 relu channels, unmasked (ATL overlaps) ----
            # off-band dt is 0, so a_k = relu(b1k) there; the ones + M2
            # channels cancel that constant exactly (host folds
            # bf16-rounded relu(b1) into the W2c9 block).
            a_full = sb.tile([128, HID * W2F], BF16)

            def a_k(k):
                return a_full[:, k * W2F:(k + 1) * W2F]

            N_SCAL = 4
            acts = []
            for k in range(N_SCAL):
                acts.append(nc.scalar.activation(
                    a_k(k), dt_ap, Act.Relu, bias=b1c(k), scale=w1c(k)))

            # ---- DVE: relu channels 5..7 (2 ts ops each) + G copies ----
            nc.vector.tensor_scalar(a_k(N_SCAL), dt_ap, w1c(N_SCAL),
                                    b1c(N_SCAL), Alu.mult, Alu.add)
            dve_relu = [nc.vector.tensor_scalar_max(a_k(N_SCAL),
                                                    a_k(N_SCAL), 0.0)]
            g_sb = sb.tile([128, 2 * NF], BF16)
            cp_gu = nc.vector.tensor_copy(g_sb[:, 0:NF], p_g[:, 0:NF])
            cp_gl = nc.vector.tensor_copy(g_sb[0:LO, NF:2 * NF],
                                          p_g[0:LO, NF:2 * NF])
            for k in range(N_SCAL + 1, HID):
                nc.vector.tensor_scalar(a_k(k), dt_ap, w1c(k), b1c(k),
                                        Alu.mult, Alu.add)
                dve_relu.append(
                    nc.vector.tensor_scalar_max(a_k(k), a_k(k), 0.0))

            # ---- banded contraction: accumulate 9 channels x 2 folds ----
            p_out = ps.tile([LBLK, CH], F32)

            def ch_up(k, lhsT, start=False):
                return nc.tensor.matmul(
                    p_out[:], lhsT[:, 0:LBLK], g_sb[:, k * CH:(k + 1) * CH],
                    start=start, stop=False)

            def ch_lo(k, lhsT, stop=False):
                return nc.tensor.matmul(
                    p_out[:], lhsT[0:LO, LBLK:W2F],
                    g_sb[0:LO, NF + k * CH:NF + (k + 1) * CH],
                    start=False, stop=stop)

            # correction channels first (ready as soon as bd + g land):
            # ones channel (block 8) then M2 = band-1 channel (block 9)
            ones_ap = t_bd[:, W2F:2 * W2F]
            m2_ap = t_bd[:, 0:W2F]
            ch_up(HID, ones_ap, start=True)
            ch_lo(HID, ones_ap)
            ch_up(HID + 1, m2_ap)
            ch_lo(HID + 1, m2_ap)
            last_pe = None
            for k in range(HID):
                ch_up(k, a_k(k))
                last_pe = ch_lo(k, a_k(k), stop=(k == HID - 1))

            # ---- row-validity fold + store ----
            o_sb = sb.tile([LBLK, CH], BF16)
            rv_mul = nc.vector.tensor_scalar_mul(o_sb[:], p_out[:], rv)
            dma_o = nc.sync.dma_start(out_d[:], o_sb[:])

            # The Tile kernel-tail drain waits on every outstanding
            # semaphore, but TRN2 instructions encode at most one sync
            # wait. Observe each producer from the SP sequencer with
            # single-wait nops so the drain itself needs none.
            for prod in (dma_fw, dma_dp, dma_bd, dma_o,
                         acts[-1], rv_mul, last_pe):
                nop = nc.sync.nop(nofuse=True, hint="predrain_observer")
                add_dep_helper(nop.ins, prod.ins, sync=True,
                               reason="pre-drain single-wait observer")

    heavy = [(nm, type(i).__name__, len(i.sync_info.on_wait))
             for nm, i in nc.inst_map.items()
             if getattr(i, "sync_info", None) is not None
             and i.sync_info.on_wait
             and len(i.sync_info.on_wait) > 1
             and type(i).__name__ != "InstDrain"]
    if heavy:
        raise RuntimeError(f"multi-wait instructions would fail walrus: {heavy}")
    return nc


def kernel(times, features, lengths, true_ids, sim_size, w1, b1, w2, b2):
    global LAST
    times = np.ascontiguousarray(np.asarray(times, dtype=np.float32))
    features = np.ascontiguousarray(np.asarray(features, dtype=np.float32))
    lengths = np.asarray(lengths)
    true_ids = np.asarray(true_ids)
    sim = int(np.asarray(sim_size))
    w1 = np.asarray(w1, dtype=np.float32).reshape(-1)
    b1 = np.asarray(b1, dtype=np.float32).reshape(-1)
    w2 = np.asarray(w2, dtype=np.float32)
    b2 = np.asarray(b2, dtype=np.float32)

    W = (sim + 1) * KS
    WIN = LBLK + W - 1
    LO = WIN - 128

    if W not in _prog_cache:
        _prog_cache[W] = _build(W)
    nc = _prog_cache[W]

    # w2 blocks [i, k*16+o], then B2 (ones channel), then W2c9 (M2 channel)
    ck = np.maximum(b1, 0).astype(NPBF16).astype(np.float32)
    W2k = w2.reshape(HID, CH, CH)
    B2m = b2.reshape(CH, CH)
    W2c9 = (ck[:, None, None] * W2k).sum(0) + B2m
    w2p = np.concatenate(
        [W2k.transpose(1, 0, 2).reshape(CH, HID * CH), B2m, W2c9],
        axis=1).astype(np.float32)

    # folded band geometry, shared across cores: band_up[jl, p] for
    # jl-p in [0, W-1]; band_lo[q, p] (jl = 128+q) for p >= 128+q-(W-1)
    jl = np.arange(128)[:, None]
    pp = np.arange(128)[None, :]
    band_up = ((pp >= jl - (W - 1)) & (pp <= jl)).astype(np.float32)
    band_lo = np.zeros((128, 128), np.float32)
    band_lo[:LO] = (pp >= (128 - (W - 1)) + jl[:LO]).astype(np.float32)
    band_f32 = np.concatenate([band_up, band_lo], axis=1)
    ones_f = np.ones((128, 128), np.float32)
    ones_lo = np.zeros((128, 128), np.float32)
    ones_lo[:LO] = 1.0
    bd_host = np.concatenate(
        [band_up - 1.0, band_lo - 1.0, ones_f, ones_lo],
        axis=1).astype(NPBF16)

    in_maps = []
    for core in range(NCORES):
        b, blk = divmod(core, NBLK)
        l0 = blk * LBLK
        idx = np.arange(l0 - (W - 1), l0 + LBLK)
        valid = idx >= 0
        idxc = np.clip(idx, 0, L - 1)
        t_win = np.where(valid, times[b, idxc], 0.0).astype(np.float32)
        tiw = (true_ids[b, idxc] & valid).astype(np.float32)
        f_mask = features[b, idxc, :] * tiw[:, None]
        t_row = times[b, l0:l0 + LBLK].astype(np.float32)
        rv = (np.arange(l0, l0 + LBLK) <=
              (sim + 1) * (int(lengths[b]) - 1)).astype(np.float32)

        # dt folded, band-masked
        dt = t_row[None, :] - t_win[:, None]        # [WIN(jl), 128(p)]
        dt_fold = np.zeros((128, W2F), np.float32)
        dt_fold[:, 0:128] = dt[0:128]
        dt_fold[:LO, 128:256] = dt[128:WIN]
        dt_fold *= band_f32

        dp = np.zeros((128, NPAR), np.float32)
        dp[:, 0:W2F] = dt_fold
        dp[:, W2F] = rv
        dp[:, W2F + 1:W2F + 1 + HID] = w1[None, :]
        dp[:, W2F + 1 + HID:W2F + 1 + 2 * HID] = b1[None, :]

        fw = np.zeros((CH, W2F + NF), np.float32)
        fw[:, :WIN] = f_mask.T
        fw[:, W2F:W2F + NF] = w2p

        in_maps.append({"dp": dp, "bd": bd_host,
                        "fw": fw.astype(NPBF16)})

    res = run_bass_kernel_spmd(nc, in_maps, core_ids=list(range(NCORES)),
                               trace=TRACE)
    LAST = res

    out = np.zeros((BS, L, CH), np.float32)
    for core in range(NCORES):
        b, blk = divmod(core, NBLK)
        out[b, blk * LBLK:(blk + 1) * LBLK, :] = \
            res.results[core]["out"].astype(np.float32)
    return out
